# revision 14
# baseline (speedup 1.0000x reference)
"""GCMCGraphConv forward on 8 trn2 NeuronCores (Bass/Tile) — v2.

reference:
    rf  = review_feat @ w_review.T                      [E, F]
    msg = (x[src] + weight[src] + rf) * ci[src]         [E, F]
    h   = segment_sum(msg, dst, N)                      [N, F]
    out = h * ci

Strategy (dst-owner sharding; dma_gather over 4 SWDGE queues):
  - Core c owns nodes [c*NPC, (c+1)*NPC) = 147 blocks of 128.  Host routes
    every edge to the owner of its dst and orders the core's edges as
    (group of M dst-blocks) -> (src window of 30000 nodes) -> (dst block),
    padding each (block, window) run to whole 128-edge tiles (counts maxed
    across cores so all 8 cores run one SPMD program).
  - Node table [x | w] is bf16 [N, 128] in DRAM; 5 windows of 30000 rows
    keep gather indices inside int16.  Each (group, window) run is fetched
    with gpsimd.dma_gather calls (<=1024 rows each) rotated across 4 SWDGE
    queues — measured ~2 ns/row vs ~11 ns/row for indirect_dma_start.
  - Per 128-edge tile, a ci-scaled one-hot S[e,n] = ci[src_e]*(dstloc_e==n)
    is built EITHER on DVE (fused is_equal*mult reading a PSUM-resident
    iota: PSUM source keeps DVE off 2-port perf mode, which would lock
    GpSimd out of SBUF and stall SWDGE descriptor generation) OR on ACT
    (t2=Square(iota-dl); oh=Relu(t2*(-ci)+ci)), split to balance engines.
  - PE per tile: psA[n,f] += S^T@x_rows ; psA += S^T@w_rows ;
    psB[k,n] += rfeat_tile^T @ S.  PSUM accumulators live across the
    group's window sweep (one PSUM bank each; M=2 keeps <=7 banks).
  - Block finalize: bt=copy(psB); psA += bt^T@w_review^T; out=psA*ci[dst].

Host does index math / layout / dtype-cast only (routing, padding,
permutation, negation of metadata streams); all feature arithmetic
(gathers, messages, sums, matmuls, scaling) runs on device.
"""

import os
import numpy as np
import ml_dtypes
from contextlib import ExitStack

import concourse.bass as bass
import concourse.tile as tile
from concourse import bacc, mybir
from concourse.bass_utils import run_bass_kernel_spmd
from concourse.library_config import mlp

P = 128
F = 64
F2 = 128
N_NODES = 150000
N_EDGES = 1250000
N_CORES = 8
WIN = 30000                 # nodes per gather window (int16-safe)
NWIN = N_NODES // WIN       # 5
M = int(os.environ.get("GCMC_M", "2"))            # dst-blocks per group
CALL_TILES = int(os.environ.get("GCMC_CT", "8"))  # max tiles per dma_gather
NQ = int(os.environ.get("GCMC_NQ", "4"))          # SWDGE queues
DVE_N = int(os.environ.get("GCMC_DVE", "5"))      # of 8 tiles on DVE
ABL = os.environ.get("GCMC_ABL", "")              # "", nogather, gatheronly
CHUNK = 16                  # tiles per rfeat chunk DMA
PAD_DL = 16000.0

bf16 = ml_dtypes.bfloat16


# --------------------------------------------------------------- host prep

def host_prep(x, weight, w_review, review_feat, ci, src, dst, n_cores):
    """Route edges to dst-owner cores; build slot-ordered DMA streams.

    Index math, layout, dtype casts only -- no feature arithmetic.
    """
    N, FF = x.shape
    NPC = N // n_cores          # 18750
    K = (NPC + P - 1) // P      # 147
    NG = (K + M - 1) // M

    owner = dst // NPC
    dloc = dst - owner * NPC
    blk = dloc >> 7
    grp = blk // M
    win = src // WIN
    loc = (src - win * WIN).astype(np.int16)

    # per-core edge sets ordered by (grp, win, blk)
    per_core = []
    cnt = np.zeros((n_cores, K, NWIN), np.int64)
    for c in range(n_cores):
        sel = np.nonzero(owner == c)[0]
        order = np.lexsort((blk[sel], win[sel], grp[sel]))
        e = sel[order]
        per_core.append(e)
        np.add.at(cnt[c], (blk[e], win[e]), 1)

    nt_kw = -(-cnt.max(axis=0) // P)        # [K, NWIN] tiles, maxed on cores
    assert nt_kw.sum(axis=1).min() >= 1

    # static tile table in slot order
    tile_k = []
    tile_w = []
    call_list = []      # (w, t0, ntiles, queue)
    t0_kw = np.zeros((K, NWIN), np.int64)
    qrot = 0
    for g in range(NG):
        ks = range(g * M, min((g + 1) * M, K))
        for w in range(NWIN):
            gw_t0 = len(tile_k)
            for k in ks:
                t0_kw[k, w] = len(tile_k)
                for _ in range(int(nt_kw[k, w])):
                    tile_k.append(k)
                    tile_w.append(w)
            ngw = len(tile_k) - gw_t0
            t = gw_t0
            while t < gw_t0 + ngw:
                nt = min(CALL_TILES, gw_t0 + ngw - t)
                call_list.append((w, t, nt, qrot % NQ))
                qrot += 1
                t += nt
    NT = len(tile_k)
    tile_k = np.asarray(tile_k)
    tile_w = np.asarray(tile_w)
    # first/last tile of each block (static)
    first_t = np.full(K, -1, np.int64)
    last_t = np.full(K, -1, np.int64)
    for t in range(NT):
        k = tile_k[t]
        if first_t[k] < 0:
            first_t[k] = t
        last_t[k] = t

    nchunks = -(-NT // CHUNK)

    # per-core streams
    table = np.zeros((N, F2), bf16)
    table[:, 0:F] = x.astype(bf16)
    table[:, F:F2] = weight.astype(bf16)
    wrT = np.ascontiguousarray(w_review.T).astype(bf16)
    iota = np.broadcast_to(np.arange(P, dtype=np.float32), (P, P))
    iota = np.ascontiguousarray(iota)

    in_maps = []
    for c in range(n_cores):
        e = per_core[c]
        kk = blk[e]
        ww = win[e]
        # rank within each contiguous (k,w) run of the sorted edge list
        key = kk * NWIN + ww
        change = np.r_[True, key[1:] != key[:-1]]
        run_start = np.maximum.accumulate(
            np.where(change, np.arange(len(e)), 0))
        rank = np.arange(len(e)) - run_start
        slot = t0_kw[kk, ww] * P + rank
        assert (rank < nt_kw[kk, ww] * P).all()

        slots_idx = np.zeros(NT * P, np.int16)
        slots_dl = np.full(NT * P, -PAD_DL, np.float32)   # dls (positive)
        slots_ci = np.zeros(NT * P, np.float32)
        slots_idx[slot] = loc[e]
        slots_dl[slot] = (dloc[e] - kk * P).astype(np.float32)
        slots_ci[slot] = ci[src[e], 0]

        # idx stream wrapped: [16, NT*8] at (s%16, (s//128)*8+(s%128)//16)
        s_all = np.arange(NT * P)
        idx16 = np.zeros((16, NT * 8), np.int16)
        idx16[s_all % 16, (s_all // P) * 8 + (s_all % P) // 16] = slots_idx
        idx16 = np.tile(idx16, (8, 1))

        dls = np.ascontiguousarray(slots_dl.reshape(NT, P).T)
        cis = np.ascontiguousarray(slots_ci.reshape(NT, P).T)
        cisneg = np.ascontiguousarray(-slots_ci.reshape(NT, P).T)
        dlneg = np.ascontiguousarray(-dls)

        # rfeat in slot order, packed [c, p, t, f] for 2KB/partition chunks
        rf = np.zeros((nchunks * P * CHUNK, FF), bf16)
        sl = slot
        ch = sl // (P * CHUNK)
        within = sl % (P * CHUNK)
        tt = within // P
        pp = within % P
        rf[ch * (P * CHUNK) + pp * CHUNK + tt] = review_feat[e].astype(bf16)

        nodes = c * NPC + np.arange(K * P)
        cic = np.zeros(K * P, np.float32)
        v = nodes < (c + 1) * NPC
        cic[v] = ci[nodes[v], 0]

        in_maps.append({
            "table": table, "wrT": wrT, "iota": iota,
            "idx16": idx16, "dls": dls, "dlneg": dlneg,
            "cis": cis, "cisneg": cisneg, "rfs": rf,
            "cic": np.ascontiguousarray(cic.reshape(K, P).T),
        })

    meta = dict(N=N, F=FF, NPC=NPC, K=K, NT=NT, n_cores=n_cores,
                nchunks=nchunks,
                tile_k=tile_k.tolist(), tile_w=tile_w.tolist(),
                first_t=first_t.tolist(), last_t=last_t.tolist(),
                call_list=call_list)
    return in_maps, meta


# ------------------------------------------------------------- bass program

def build_program(meta, reps=1):
    N = meta["N"]; NPC = meta["NPC"]; K = meta["K"]; NT = meta["NT"]
    nchunks = meta["nchunks"]; n_cores = meta["n_cores"]
    tile_k = meta["tile_k"]; tile_w = meta["tile_w"]
    first_t = meta["first_t"]; last_t = meta["last_t"]
    call_list = meta["call_list"]
    dt = mybir.dt

    call_at = {t0: (w, nt, q) for (w, t0, nt, q) in call_list}

    nc = bacc.Bacc("TRN2", target_bir_lowering=False, debug=False,
                   enable_asserts=False, num_devices=n_cores,
                   num_swdge_queues=NQ)

    table = nc.dram_tensor("table", [N, F2], dt.bfloat16,
                           kind="ExternalInput").ap()
    wrT = nc.dram_tensor("wrT", [F, F], dt.bfloat16,
                         kind="ExternalInput").ap()
    iota = nc.dram_tensor("iota", [P, P], dt.float32,
                          kind="ExternalInput").ap()
    idx16 = nc.dram_tensor("idx16", [P, NT * 8], dt.int16,
                           kind="ExternalInput").ap()
    dls = nc.dram_tensor("dls", [P, NT], dt.float32,
                         kind="ExternalInput").ap()
    dlneg = nc.dram_tensor("dlneg", [P, NT], dt.float32,
                           kind="ExternalInput").ap()
    cis = nc.dram_tensor("cis", [P, NT], dt.float32,
                         kind="ExternalInput").ap()
    cisneg = nc.dram_tensor("cisneg", [P, NT], dt.float32,
                            kind="ExternalInput").ap()
    rfs = nc.dram_tensor("rfs", [nchunks * P * CHUNK, F], dt.bfloat16,
                         kind="ExternalInput").ap()
    cic = nc.dram_tensor("cic", [P, K], dt.float32,
                         kind="ExternalInput").ap()
    out = nc.dram_tensor("out", [NPC, F], dt.float32,
                         kind="ExternalOutput").ap()

    rf_view = rfs.rearrange("(c p t) f -> c p (t f)", p=P, t=CHUNK)

    with tile.TileContext(nc) as tc, ExitStack() as ctx:
        nc.gpsimd.load_library(mlp)
        consts = ctx.enter_context(tc.tile_pool(name="consts", bufs=1))
        mpool = ctx.enter_context(tc.tile_pool(name="meta", bufs=2))
        gpool = ctx.enter_context(tc.tile_pool(name="gather", bufs=8))
        rfpool = ctx.enter_context(tc.tile_pool(name="rfeat", bufs=4))
        ohpool = ctx.enter_context(tc.tile_pool(name="onehot", bufs=16))
        t2pool = ctx.enter_context(tc.tile_pool(name="tsq", bufs=8))
        opool = ctx.enter_context(tc.tile_pool(name="outs", bufs=4))
        btpool = ctx.enter_context(tc.tile_pool(name="btile", bufs=3))
        psa = ctx.enter_context(tc.tile_pool(name="psa", bufs=3,
                                             space="PSUM"))
        psb = ctx.enter_context(tc.tile_pool(name="psb", bufs=3,
                                             space="PSUM"))
        psi = ctx.enter_context(tc.tile_pool(name="psi", bufs=1,
                                             space="PSUM"))

        iota_sb = consts.tile([P, P], dt.float32, tag="iota")
        nc.sync.dma_start(out=iota_sb[:], in_=iota[:])
        wrT_sb = consts.tile([F, F], dt.bfloat16, tag="wrT")
        nc.sync.dma_start(out=wrT_sb[:], in_=wrT[:])
        cic_sb = consts.tile([P, K], dt.float32, tag="cic")
        nc.sync.dma_start(out=cic_sb[:], in_=cic[:])
        iota_ps = psi.tile([P, P], dt.float32, tag="iops")
        nc.vector.tensor_copy(iota_ps[:], iota_sb[:])

        def body(iv=None):
            idx_sb = mpool.tile([P, NT * 8], dt.int16, tag="idx")
            nc.sync.dma_start(out=idx_sb[:], in_=idx16[:])
            dls_sb = mpool.tile([P, NT], dt.float32, tag="dls")
            nc.sync.dma_start(out=dls_sb[:], in_=dls[:])
            dln_sb = mpool.tile([P, NT], dt.float32, tag="dln")
            nc.sync.dma_start(out=dln_sb[:], in_=dlneg[:])
            cis_sb = mpool.tile([P, NT], dt.float32, tag="cis")
            nc.sync.dma_start(out=cis_sb[:], in_=cis[:])
            cin_sb = mpool.tile([P, NT], dt.float32, tag="cin")
            nc.sync.dma_start(out=cin_sb[:], in_=cisneg[:])
            psA = {}
            psB = {}
            g_t = None
            g_t0 = 0
            rfc = None
            for t in range(NT):
                k = tile_k[t]
                w = tile_w[t]
                first = (t == first_t[k])
                last = (t == last_t[k])
                if t in call_at:
                    cw, cnt_t, q = call_at[t]
                    g_t0 = t
                    g_t = gpool.tile([P, CALL_TILES, F2], dt.bfloat16,
                                     tag="g")
                    nidx = cnt_t * P
                    if ABL == "nogather":
                        base = (t % 200) * P
                        nc.sync.dma_start(
                            out=g_t[:, 0:cnt_t, :],
                            in_=table[base:base + cnt_t * P, :].rearrange(
                                "(j p) w -> p j w", j=cnt_t, p=P))
                    else:
                        nc.gpsimd.dma_gather(
                            out_ap=g_t[:, 0:cnt_t, :],
                            in_ap=table[cw * WIN:(cw + 1) * WIN, :],
                            idxs_ap=idx_sb[:, t * 8:(t + cnt_t) * 8],
                            num_idxs=nidx, num_idxs_reg=nidx, elem_size=F2,
                            queue_num=q)
                if t % CHUNK == 0:
                    cwid = min(CHUNK, NT - t)
                    rfc = rfpool.tile([P, CHUNK * F], dt.bfloat16,
                                      tag="rfc")
                    nc.sync.dma_start(out=rfc[:, :cwid * F],
                                      in_=rf_view[t // CHUNK][:, :cwid * F])

                if ABL == "gatheronly":
                    continue
                if first:
                    pa = psa.tile([P, F], dt.float32, tag="psA")
                    pb = psb.tile([F, P], dt.float32, tag="psB")
                    psA[k], psB[k] = pa, pb

                oh = ohpool.tile([P, P], dt.bfloat16, tag="oh")
                if (t % 8) < DVE_N:
                    # DVE: S = (iota==dl)*ci, PSUM-source avoids 2-port mode
                    nc.vector.tensor_scalar(
                        out=oh[:], in0=iota_ps[:],
                        scalar1=dls_sb[:, t:t + 1],
                        scalar2=cis_sb[:, t:t + 1],
                        op0=mybir.AluOpType.is_equal,
                        op1=mybir.AluOpType.mult)
                else:
                    # ACT: t2=(iota-dl)^2 ; S=Relu(t2*(-ci)+ci)
                    t2 = t2pool.tile([P, P], dt.bfloat16, tag="t2")
                    nc.scalar.activation(
                        t2[:], iota_sb[:],
                        mybir.ActivationFunctionType.Square,
                        bias=dln_sb[:, t:t + 1], scale=1.0)
                    nc.scalar.activation(
                        oh[:], t2[:],
                        mybir.ActivationFunctionType.Relu,
                        bias=cis_sb[:, t:t + 1],
                        scale=cin_sb[:, t:t + 1])

                gj = t - g_t0
                nc.tensor.matmul(psA[k][:], lhsT=oh[:],
                                 rhs=g_t[:, gj, 0:F],
                                 start=first, stop=False)
                nc.tensor.matmul(psA[k][:], lhsT=oh[:],
                                 rhs=g_t[:, gj, F:F2],
                                 start=False, stop=False)
                tl = t % CHUNK
                nc.tensor.matmul(psB[k][:], lhsT=rfc[:, tl * F:tl * F + F],
                                 rhs=oh[:], start=first, stop=last)

                if last:
                    bt = btpool.tile([F, P], dt.bfloat16, tag="bt")
                    nc.scalar.copy(bt[:], psB[k][:])
                    nc.tensor.matmul(psA[k][:], lhsT=bt[:], rhs=wrT_sb[:],
                                     start=False, stop=True)
                    o2 = opool.tile([P, F], dt.float32, tag="o2")
                    nc.vector.tensor_scalar_mul(o2[:], psA[k][:],
                                                cic_sb[:, k:k + 1])
                    rows = min(P, NPC - k * P)
                    nc.sync.dma_start(out=out[k * P:k * P + rows, :],
                                      in_=o2[:rows, :])
                    del psA[k], psB[k]

        if reps == 1:
            body()
        else:
            with tc.For_i(0, reps, 1) as iv:
                body(iv)

    nc.compile()
    return nc


# ------------------------------------------------------------------ driver

_CACHE = {}


def _get_program(meta, reps=1):
    key = (meta["N"], meta["NPC"], meta["K"], meta["NT"], meta["n_cores"],
           tuple(meta["tile_k"]), tuple(meta["tile_w"]),
           tuple(tuple(x) for x in meta["call_list"]), reps, ABL, DVE_N)
    if key not in _CACHE:
        _CACHE[key] = build_program(meta, reps=reps)
    return _CACHE[key]


def run(inputs, n_cores=N_CORES, trace=False, reps=1):
    in_maps, meta = host_prep(
        inputs["x"], inputs["weight"], inputs["w_review"],
        inputs["review_feat"], inputs["ci"], inputs["src"], inputs["dst"],
        n_cores)
    nc = _get_program(meta, reps=reps)
    res = run_bass_kernel_spmd(nc, in_maps, list(range(n_cores)),
                               trace=trace)
    outp = np.concatenate([res.results[c]["out"] for c in range(n_cores)],
                          axis=0)
    return outp, res


def kernel(**inputs) -> np.ndarray:
    inputs = {k: np.asarray(v) for k, v in inputs.items()}
    last = None
    for attempt in range(3):
        try:
            outp, _ = run(inputs, n_cores=N_CORES)
            return outp
        except Exception as e:          # transient accelerator errors
            last = e
    raise last


# revision 18
# speedup vs baseline: 1.4282x; 1.4282x over previous
"""GCMCGraphConv forward on 8 trn2 NeuronCores (Bass/Tile) — v2.

reference:
    rf  = review_feat @ w_review.T                      [E, F]
    msg = (x[src] + weight[src] + rf) * ci[src]         [E, F]
    h   = segment_sum(msg, dst, N)                      [N, F]
    out = h * ci

Strategy (dst-owner sharding; dma_gather over 4 SWDGE queues):
  - Core c owns nodes [c*NPC, (c+1)*NPC) = 147 blocks of 128.  Host routes
    every edge to the owner of its dst and orders the core's edges as
    (group of M dst-blocks) -> (src window of 30000 nodes) -> (dst block),
    padding each (block, window) run to whole 128-edge tiles (counts maxed
    across cores so all 8 cores run one SPMD program).
  - Node table [x | w] is bf16 [N, 128] in DRAM; 5 windows of 30000 rows
    keep gather indices inside int16.  Each (group, window) run is fetched
    with gpsimd.dma_gather calls (<=1024 rows each) rotated across 4 SWDGE
    queues — measured ~2 ns/row vs ~11 ns/row for indirect_dma_start.
  - Per 128-edge tile, a ci-scaled one-hot S[e,n] = ci[src_e]*(dstloc_e==n)
    is built EITHER on DVE (fused is_equal*mult reading a PSUM-resident
    iota: PSUM source keeps DVE off 2-port perf mode, which would lock
    GpSimd out of SBUF and stall SWDGE descriptor generation) OR on ACT
    (t2=Square(iota-dl); oh=Relu(t2*(-ci)+ci)), split to balance engines.
  - PE per tile: psA[n,f] += S^T@x_rows ; psA += S^T@w_rows ;
    psB[k,n] += rfeat_tile^T @ S.  PSUM accumulators live across the
    group's window sweep (one PSUM bank each; M=2 keeps <=7 banks).
  - Block finalize: bt=copy(psB); psA += bt^T@w_review^T; out=psA*ci[dst].

Host does index math / layout / dtype-cast only (routing, padding,
permutation, negation of metadata streams); all feature arithmetic
(gathers, messages, sums, matmuls, scaling) runs on device.
"""

import os
import numpy as np
import ml_dtypes
from contextlib import ExitStack

import concourse.bass as bass
import concourse.tile as tile
from concourse import bacc, mybir
from concourse.bass_utils import run_bass_kernel_spmd
from concourse.library_config import mlp

P = 128
F = 64
F2 = 128
N_NODES = 150000
N_EDGES = 1250000
N_CORES = 8
WIN = 30000                 # nodes per gather window (int16-safe)
NWIN = N_NODES // WIN       # 5
M = int(os.environ.get("GCMC_M", "4"))            # dst-blocks per group
CALL_TILES = int(os.environ.get("GCMC_CT", "8"))  # max tiles per dma_gather
NQ = int(os.environ.get("GCMC_NQ", "4"))          # SWDGE queues
DVE_N = int(os.environ.get("GCMC_DVE", "5"))      # of 8 tiles on DVE
ABL = os.environ.get("GCMC_ABL", "")              # "", nogather, gatheronly
CHUNK = 16                  # tiles per rfeat chunk DMA
PAD_DL = 16000.0

bf16 = ml_dtypes.bfloat16


# --------------------------------------------------------------- host prep

def host_prep(x, weight, w_review, review_feat, ci, src, dst, n_cores):
    """Route edges to dst-owner cores; build slot-ordered DMA streams.

    Index math, layout, dtype casts only -- no feature arithmetic.
    """
    N, FF = x.shape
    NPC = N // n_cores          # 18750
    K = (NPC + P - 1) // P      # 147
    NG = (K + M - 1) // M

    owner = dst // NPC
    dloc = dst - owner * NPC
    blk = dloc >> 7
    grp = blk // M
    win = src // WIN
    loc = (src - win * WIN).astype(np.int16)

    # per-core edge sets ordered by (grp, win, blk)
    per_core = []
    cnt = np.zeros((n_cores, K, NWIN), np.int64)
    for c in range(n_cores):
        sel = np.nonzero(owner == c)[0]
        order = np.lexsort((blk[sel], win[sel], grp[sel]))
        e = sel[order]
        per_core.append(e)
        np.add.at(cnt[c], (blk[e], win[e]), 1)

    nt_kw = -(-cnt.max(axis=0) // P)        # [K, NWIN] tiles, maxed on cores
    assert nt_kw.sum(axis=1).min() >= 1

    # static tile table in slot order
    tile_k = []
    tile_w = []
    call_list = []      # (w, t0, ntiles, queue)
    t0_kw = np.zeros((K, NWIN), np.int64)
    qrot = 0
    for g in range(NG):
        ks = range(g * M, min((g + 1) * M, K))
        for w in range(NWIN):
            gw_t0 = len(tile_k)
            for k in ks:
                t0_kw[k, w] = len(tile_k)
                for _ in range(int(nt_kw[k, w])):
                    tile_k.append(k)
                    tile_w.append(w)
            ngw = len(tile_k) - gw_t0
            ncall = -(-ngw // CALL_TILES)
            per = -(-ngw // max(ncall, 1))
            t = gw_t0
            while t < gw_t0 + ngw:
                nt = min(per, gw_t0 + ngw - t)
                call_list.append((w, t, nt, qrot % NQ))
                qrot += 1
                t += nt
    NT = len(tile_k)
    tile_k = np.asarray(tile_k)
    tile_w = np.asarray(tile_w)
    # first/last tile of each block (static)
    first_t = np.full(K, -1, np.int64)
    last_t = np.full(K, -1, np.int64)
    for t in range(NT):
        k = tile_k[t]
        if first_t[k] < 0:
            first_t[k] = t
        last_t[k] = t

    nchunks = -(-NT // CHUNK)
    grp_first = []
    grp_last = []
    for g in range(NG):
        ks = list(range(g * M, min((g + 1) * M, K)))
        grp_first.append(min(first_t[k] for k in ks))
        grp_last.append(max(last_t[k] for k in ks))

    # per-core streams
    table = np.zeros((N, F2), bf16)
    table[:, 0:F] = x.astype(bf16)
    table[:, F:F2] = weight.astype(bf16)
    wrT = np.ascontiguousarray(w_review.T).astype(bf16)
    iota = np.broadcast_to(np.arange(P, dtype=np.float32), (P, P))
    iota = np.ascontiguousarray(iota)

    in_maps = []
    for c in range(n_cores):
        e = per_core[c]
        kk = blk[e]
        ww = win[e]
        # rank within each contiguous (k,w) run of the sorted edge list
        key = kk * NWIN + ww
        change = np.r_[True, key[1:] != key[:-1]]
        run_start = np.maximum.accumulate(
            np.where(change, np.arange(len(e)), 0))
        rank = np.arange(len(e)) - run_start
        slot = t0_kw[kk, ww] * P + rank
        assert (rank < nt_kw[kk, ww] * P).all()

        slots_idx = np.zeros(NT * P, np.int16)
        slots_dl = np.full(NT * P, -PAD_DL, np.float32)   # dls (positive)
        slots_ci = np.zeros(NT * P, np.float32)
        slots_idx[slot] = loc[e]
        slots_dl[slot] = (dloc[e] - kk * P).astype(np.float32)
        slots_ci[slot] = ci[src[e], 0]

        # idx stream wrapped: [16, NT*8] at (s%16, (s//128)*8+(s%128)//16)
        s_all = np.arange(NT * P)
        idx16 = np.zeros((16, NT * 8), np.int16)
        idx16[s_all % 16, (s_all // P) * 8 + (s_all % P) // 16] = slots_idx
        idx16 = np.tile(idx16, (8, 1))

        dls = np.ascontiguousarray(slots_dl.reshape(NT, P).T)
        cis = np.ascontiguousarray(slots_ci.reshape(NT, P).T)
        cisneg = np.ascontiguousarray(-slots_ci.reshape(NT, P).T)
        dlneg = np.ascontiguousarray(-dls)

        # rfeat in slot order, packed [c, p, t, f] for 2KB/partition chunks
        rf = np.zeros((nchunks * P * CHUNK, FF), bf16)
        sl = slot
        ch = sl // (P * CHUNK)
        within = sl % (P * CHUNK)
        tt = within // P
        pp = within % P
        rf[ch * (P * CHUNK) + pp * CHUNK + tt] = review_feat[e].astype(bf16)

        nodes = c * NPC + np.arange(K * P)
        cic = np.zeros(K * P, np.float32)
        v = nodes < (c + 1) * NPC
        cic[v] = ci[nodes[v], 0]

        in_maps.append({
            "table": table, "wrT": wrT, "iota": iota,
            "idx16": idx16, "dls": dls, "dlneg": dlneg,
            "cis": cis, "cisneg": cisneg, "rfs": rf,
            "cic": np.ascontiguousarray(cic.reshape(K, P).T),
        })

    meta = dict(N=N, F=FF, NPC=NPC, K=K, NT=NT, n_cores=n_cores,
                nchunks=nchunks,
                tile_k=tile_k.tolist(), tile_w=tile_w.tolist(),
                first_t=first_t.tolist(), last_t=last_t.tolist(),
                grp_first=grp_first, grp_last=grp_last,
                call_list=call_list)
    return in_maps, meta


# ------------------------------------------------------------- bass program

def build_program(meta, reps=1):
    N = meta["N"]; NPC = meta["NPC"]; K = meta["K"]; NT = meta["NT"]
    nchunks = meta["nchunks"]; n_cores = meta["n_cores"]
    tile_k = meta["tile_k"]; tile_w = meta["tile_w"]
    first_t = meta["first_t"]; last_t = meta["last_t"]
    grp_first = meta["grp_first"]; grp_last = meta["grp_last"]
    call_list = meta["call_list"]
    dt = mybir.dt
    grp_first_set = {t: g for g, t in enumerate(grp_first)}
    grp_last_set = {t: g for g, t in enumerate(grp_last)}

    call_at = {t0: (w, nt, q) for (w, t0, nt, q) in call_list}

    nc = bacc.Bacc("TRN2", target_bir_lowering=False, debug=False,
                   enable_asserts=False, num_devices=n_cores,
                   num_swdge_queues=NQ)

    table = nc.dram_tensor("table", [N, F2], dt.bfloat16,
                           kind="ExternalInput").ap()
    wrT = nc.dram_tensor("wrT", [F, F], dt.bfloat16,
                         kind="ExternalInput").ap()
    iota = nc.dram_tensor("iota", [P, P], dt.float32,
                          kind="ExternalInput").ap()
    idx16 = nc.dram_tensor("idx16", [P, NT * 8], dt.int16,
                           kind="ExternalInput").ap()
    dls = nc.dram_tensor("dls", [P, NT], dt.float32,
                         kind="ExternalInput").ap()
    dlneg = nc.dram_tensor("dlneg", [P, NT], dt.float32,
                           kind="ExternalInput").ap()
    cis = nc.dram_tensor("cis", [P, NT], dt.float32,
                         kind="ExternalInput").ap()
    cisneg = nc.dram_tensor("cisneg", [P, NT], dt.float32,
                            kind="ExternalInput").ap()
    rfs = nc.dram_tensor("rfs", [nchunks * P * CHUNK, F], dt.bfloat16,
                         kind="ExternalInput").ap()
    cic = nc.dram_tensor("cic", [P, K], dt.float32,
                         kind="ExternalInput").ap()
    out = nc.dram_tensor("out", [NPC, F], dt.float32,
                         kind="ExternalOutput").ap()

    rf_view = rfs.rearrange("(c p t) f -> c p (t f)", p=P, t=CHUNK)

    with tile.TileContext(nc) as tc, ExitStack() as ctx:
        nc.gpsimd.load_library(mlp)
        consts = ctx.enter_context(tc.tile_pool(name="consts", bufs=1))
        mpool = ctx.enter_context(tc.tile_pool(name="meta", bufs=2))
        gpool = ctx.enter_context(tc.tile_pool(name="gather", bufs=8))
        rfpool = ctx.enter_context(tc.tile_pool(name="rfeat", bufs=4))
        ohpool = ctx.enter_context(tc.tile_pool(name="onehot", bufs=16))
        t2pool = ctx.enter_context(tc.tile_pool(name="tsq", bufs=8))
        opool = ctx.enter_context(tc.tile_pool(name="outs", bufs=8))
        btpool = ctx.enter_context(tc.tile_pool(name="btile", bufs=8))
        psa = ctx.enter_context(tc.tile_pool(name="psa", bufs=3,
                                             space="PSUM"))
        psb = ctx.enter_context(tc.tile_pool(name="psb", bufs=3,
                                             space="PSUM"))
        psi = ctx.enter_context(tc.tile_pool(name="psi", bufs=1,
                                             space="PSUM"))

        iota_sb = consts.tile([P, P], dt.float32, tag="iota")
        nc.sync.dma_start(out=iota_sb[:], in_=iota[:])
        wrT_sb = consts.tile([F, F], dt.bfloat16, tag="wrT")
        nc.sync.dma_start(out=wrT_sb[:], in_=wrT[:])
        cic_sb = consts.tile([P, K], dt.float32, tag="cic")
        nc.sync.dma_start(out=cic_sb[:], in_=cic[:])
        iota_ps = psi.tile([P, P], dt.float32, tag="iops")
        nc.vector.tensor_copy(iota_ps[:], iota_sb[:])
        dmy = consts.tile([P, M * P], dt.bfloat16, tag="dmy")
        nc.vector.memset(dmy[:], 0.0)

        def body(iv=None):
            idx_sb = mpool.tile([P, NT * 8], dt.int16, tag="idx")
            nc.sync.dma_start(out=idx_sb[:], in_=idx16[:])
            dls_sb = mpool.tile([P, NT], dt.float32, tag="dls")
            nc.sync.dma_start(out=dls_sb[:], in_=dls[:])
            dln_sb = mpool.tile([P, NT], dt.float32, tag="dln")
            nc.sync.dma_start(out=dln_sb[:], in_=dlneg[:])
            cis_sb = mpool.tile([P, NT], dt.float32, tag="cis")
            nc.sync.dma_start(out=cis_sb[:], in_=cis[:])
            cin_sb = mpool.tile([P, NT], dt.float32, tag="cin")
            nc.sync.dma_start(out=cin_sb[:], in_=cisneg[:])
            psA = psB = None           # group-shared PSUM tiles
            gfirst = False
            g_t = None
            g_t0 = 0
            rfc = None
            for t in range(NT):
                k = tile_k[t]
                w = tile_w[t]
                ok = k % M              # block offset within group
                if t in call_at:
                    cw, cnt_t, q = call_at[t]
                    g_t0 = t
                    g_t = gpool.tile([P, CALL_TILES, F2], dt.bfloat16,
                                     tag="g")
                    nidx = cnt_t * P
                    if ABL == "nogather":
                        base = (t % 200) * P
                        nc.sync.dma_start(
                            out=g_t[:, 0:cnt_t, :],
                            in_=table[base:base + cnt_t * P, :].rearrange(
                                "(j p) w -> p j w", j=cnt_t, p=P))
                    else:
                        nc.gpsimd.dma_gather(
                            out_ap=g_t[:, 0:cnt_t, :],
                            in_ap=table[cw * WIN:(cw + 1) * WIN, :],
                            idxs_ap=idx_sb[:, t * 8:(t + cnt_t) * 8],
                            num_idxs=nidx, num_idxs_reg=nidx, elem_size=F2,
                            queue_num=q)
                if t % CHUNK == 0:
                    cwid = min(CHUNK, NT - t)
                    rfc = rfpool.tile([P, CHUNK * F], dt.bfloat16,
                                      tag="rfc")
                    nc.sync.dma_start(out=rfc[:, :cwid * F],
                                      in_=rf_view[t // CHUNK][:, :cwid * F])

                if ABL == "gatheronly":
                    continue
                if t in grp_first_set:
                    psA = psa.tile([P, M * F], dt.float32, tag="psA")
                    psB = psb.tile([F, M * P], dt.float32, tag="psB")
                    nc.tensor.matmul(psA[:], lhsT=dmy[:, 0:P],
                                     rhs=dmy[:, 0:M * F],
                                     start=True, stop=False)
                    nc.tensor.matmul(psB[:], lhsT=dmy[:, 0:F],
                                     rhs=dmy[:, 0:M * P],
                                     start=True, stop=False)

                oh = ohpool.tile([P, P], dt.bfloat16, tag="oh")
                if (t % 8) < DVE_N:
                    # DVE: S = (iota==dl)*ci, PSUM-source avoids 2-port mode
                    nc.vector.tensor_scalar(
                        out=oh[:], in0=iota_ps[:],
                        scalar1=dls_sb[:, t:t + 1],
                        scalar2=cis_sb[:, t:t + 1],
                        op0=mybir.AluOpType.is_equal,
                        op1=mybir.AluOpType.mult)
                else:
                    # ACT: t2=(iota-dl)^2 ; S=Relu(t2*(-ci)+ci)
                    t2 = t2pool.tile([P, P], dt.bfloat16, tag="t2")
                    nc.scalar.activation(
                        t2[:], iota_sb[:],
                        mybir.ActivationFunctionType.Square,
                        bias=dln_sb[:, t:t + 1], scale=1.0)
                    nc.scalar.activation(
                        oh[:], t2[:],
                        mybir.ActivationFunctionType.Relu,
                        bias=cis_sb[:, t:t + 1],
                        scale=cin_sb[:, t:t + 1])

                gj = t - g_t0
                nc.tensor.matmul(psA[:, ok * F:(ok + 1) * F], lhsT=oh[:],
                                 rhs=g_t[:, gj, 0:F],
                                 start=False, stop=False)
                nc.tensor.matmul(psA[:, ok * F:(ok + 1) * F], lhsT=oh[:],
                                 rhs=g_t[:, gj, F:F2],
                                 start=False, stop=False)
                tl = t % CHUNK
                nc.tensor.matmul(psB[:, ok * P:(ok + 1) * P],
                                 lhsT=rfc[:, tl * F:tl * F + F],
                                 rhs=oh[:], start=False,
                                 stop=(t in grp_last_set))

                if t in grp_last_set:
                    g = grp_last_set[t]
                    ks = [kk for kk in range(g * M, min((g + 1) * M, K))]
                    bts = {}
                    for kk in ks:
                        okk = kk % M
                        bt = btpool.tile([F, P], dt.bfloat16, tag="bt")
                        nc.scalar.copy(bt[:], psB[:, okk * P:(okk + 1) * P])
                        nc.tensor.matmul(psA[:, okk * F:(okk + 1) * F],
                                         lhsT=bt[:], rhs=wrT_sb[:],
                                         start=False, stop=(kk == ks[-1]))
                    for kk in ks:
                        okk = kk % M
                        o2 = opool.tile([P, F], dt.float32, tag="o2")
                        nc.vector.tensor_scalar_mul(
                            o2[:], psA[:, okk * F:(okk + 1) * F],
                            cic_sb[:, kk:kk + 1])
                        rows = min(P, NPC - kk * P)
                        nc.sync.dma_start(out=out[kk * P:kk * P + rows, :],
                                          in_=o2[:rows, :])

        if reps == 1:
            body()
        else:
            with tc.For_i(0, reps, 1) as iv:
                body(iv)

    nc.compile()
    return nc


# ------------------------------------------------------------------ driver

_CACHE = {}


def _get_program(meta, reps=1):
    key = (meta["N"], meta["NPC"], meta["K"], meta["NT"], meta["n_cores"],
           tuple(meta["tile_k"]), tuple(meta["tile_w"]),
           tuple(tuple(x) for x in meta["call_list"]), reps, ABL, DVE_N)
    if key not in _CACHE:
        _CACHE[key] = build_program(meta, reps=reps)
    return _CACHE[key]


def run(inputs, n_cores=N_CORES, trace=False, reps=1):
    in_maps, meta = host_prep(
        inputs["x"], inputs["weight"], inputs["w_review"],
        inputs["review_feat"], inputs["ci"], inputs["src"], inputs["dst"],
        n_cores)
    nc = _get_program(meta, reps=reps)
    res = run_bass_kernel_spmd(nc, in_maps, list(range(n_cores)),
                               trace=trace)
    outp = np.concatenate([res.results[c]["out"] for c in range(n_cores)],
                          axis=0)
    return outp, res


def kernel(**inputs) -> np.ndarray:
    inputs = {k: np.asarray(v) for k, v in inputs.items()}
    last = None
    for attempt in range(3):
        try:
            outp, _ = run(inputs, n_cores=N_CORES)
            return outp
        except Exception as e:          # transient accelerator errors
            last = e
    raise last
